# revision 27
# baseline (speedup 1.0000x reference)
"""Trainium2 Bass kernel for nn_PointPredictor (Molmo point-predictor head).

Strategy
--------
Both heavy ops are row-parallel [R, 2048] @ [2048, 256] matmuls:
  * subpatch_k: vit_features rows (B*N*P = 9216 rows)
  * patch_k:    the B*N = 2304 compacted/selected rows of x (the masked
    compaction is a pure gather, so only the selected rows are computed)
Rows are sharded evenly across the 8 NeuronCores (1152 + 288 = 1440 rows
per core); the small weights are replicated. The host computes the
gather indices / positions / cos-sin rotary tables (tiny mask math),
transposes the row blocks so the contraction dim lands on SBUF
partitions, and assembles/re-transposes the outputs.

On device (per core, per K-chunk of 128): the weight chunk halves
[128, 128] are the PE stationary operand and the 1440 rows stream as the
moving operand, producing transposed outputs [dp_half, row] in two
3-bank PSUM tiles (fp32 accumulation over 16 chunks). Operands stream
as fp16 (IN_DTYPE); bias lands as a K=1 rank-1 matmul; rotary runs on
the vector engine in fp32 in the transposed layout (host supplies
transposed cos/sin tables). Outputs DMA out as [dp, row] and the host
transposes back while assembling.

PSUM bank note: start=True clears has_written for the WHOLE 2KB bank,
so only the very first matmul into each bank uses start=True (explicitly
ordered first); later writers to other column ranges of the bank rely on
per-element has_written bits (overwrite-on-first-touch, accumulate
after).
"""

import numpy as np

B, S, N, P, D, DP, DV = 4, 2048, 576, 4, 2048, 256, 2048
THETA = 10000.0
NCORES = 8
RA = (B * N * P) // NCORES      # 1152 subpatch rows per core
RB = (B * N) // NCORES          # 288 patch rows per core
RT = RA + RB                    # 1440 rows per core
KC = D // 128                   # 16 contraction chunks
HALF = DP // 2                  # 128

# moving-operand ranges: A rows in <=512 slices (one PSUM bank each),
# then the B rows (1152:1440, inside bank 2)
MOVES = [("A", 0, 512), ("A", 512, 1024), ("A", 1024, 1152), ("B", 1152, 1440)]

_CACHE = {}
LAST_RESULTS = None  # BassKernelResults of the most recent run (for test.py)

# Input dtype for the matmul operands (row data + weights).
# "f16": half the DMA traffic, ~3e-4 scaled error. "f32r": ~1.5e-4 error.
IN_DTYPE = "f16"


def _build_nc():
    import concourse.bacc as bacc
    import concourse.mybir as mybir
    import concourse.tile as tile
    from concourse.tile_rust import add_dep_helper

    f32 = mybir.dt.float32
    fin = mybir.dt.float16 if IN_DTYPE == "f16" else mybir.dt.float32r

    nc = bacc.Bacc("TRN2", target_bir_lowering=False, debug=False)
    rowsT = nc.dram_tensor("rowsT", [KC, 128, RT], fin, kind="ExternalInput").ap()
    wcT = nc.dram_tensor("wcT", [KC, 128, 2 * DP], fin, kind="ExternalInput").ap()
    biasf = nc.dram_tensor("biasf", [1, 2 * DP], fin, kind="ExternalInput").ap()
    cosTt = nc.dram_tensor("cosTt", [128, RB], f32, kind="ExternalInput").ap()
    sinTt = nc.dram_tensor("sinTt", [128, RB], f32, kind="ExternalInput").ap()
    subT_out = nc.dram_tensor("subT_out", [2, 128, RA], fin, kind="ExternalOutput").ap()
    pkT_out = nc.dram_tensor("pkT_out", [2, 128, RB], f32, kind="ExternalOutput").ap()
    warm_out = nc.dram_tensor("warm_out", [1, 1], f32, kind="ExternalOutput").ap()

    PSW = 1536  # 3 banks of 512 fp32, covers the 1440 row columns

    with tile.TileContext(nc) as tc:
        with (
            tc.tile_pool(name="const", bufs=1) as kpool,
            tc.tile_pool(name="chunks", bufs=6) as cpool,
            tc.tile_pool(name="psacc", bufs=1, space="PSUM") as ppool,
            tc.tile_pool(name="outs", bufs=2) as opool,
        ):
            # transposed output accumulators: ps[h][dp_lo+p, row]
            psL = ppool.tile([128, PSW], f32, name="psL", tag="psL")
            psH = ppool.tile([128, PSW], f32, name="psH", tag="psH")
            ps = [psL, psH]

            ones_sb = kpool.tile([1, 512], fin, tag="ones")
            nc.vector.memset(ones_sb[:], 1.0)
            bias_sb = kpool.tile([1, 2 * DP], fin, tag="biasf")
            cos_sb = kpool.tile([128, RB], f32, tag="cos")
            sin_sb = kpool.tile([128, RB], f32, tag="sin")

            # Warm-up matmuls: ~3.4us of PE activity releases the HAM clock
            # gate (1.2 -> 2.4 GHz) before the first real matmul arrives.
            ps_warm = ppool.tile([128, 512], f32, name="ps_warm", tag="ps_warm")
            for w in range(14):
                nc.tensor.matmul(ps_warm[0:128, 0:128], ones_sb[0:1, 0:128],
                                 ones_sb[0:1, 0:128], start=True, stop=True)
            # ACT copy doubles as the ps_warm reader (anti-DCE) and preloads
            # the scalar engine's activation table before the epilogue copy
            junk_sb = kpool.tile([1, 1], f32, tag="junk")
            nc.scalar.copy(junk_sb[:], ps_warm[0:1, 0:1])
            nc.sync.dma_start(warm_out, junk_sb[:])

            # Per-chunk DMAs, alternating between the two HWDGE rings
            # (sync + scalar); rows and weights take opposite rings so each
            # chunk's pair lands in parallel. Constants queue after all
            # chunk loads (they are only needed by the tail).
            rt_ap = {}
            wc_ap = {}
            for c in range(KC):
                rte = nc.sync if c % 2 == 0 else nc.scalar
                wce = nc.scalar if c % 2 == 0 else nc.sync
                t = cpool.tile([128, RT], fin, name=f"rtc{c}", tag="rt", bufs=8)
                rte.dma_start(t[:], rowsT[c])
                rt_ap[c] = t[:]
                w = cpool.tile([128, 2 * DP], fin, name=f"wcc{c}", tag="wc", bufs=8)
                wce.dma_start(w[:], wcT[c])
                wc_ap[c] = w[:]
            nc.scalar.dma_start(bias_sb[:], biasf)
            nc.scalar.dma_start(cos_sb[:], cosTt)
            nc.scalar.dma_start(sin_sb[:], sinTt)

            bank_clear = {}
            for c in range(KC):
                if c == 12:
                    # biases as K=1 rank-1 accumulation (bias_half ⊗ ones),
                    # slotted mid-chain after the bias DMA has landed
                    for h in range(2):
                        for kind, r0, r1 in MOVES:
                            bcol = h * 128 if kind == "A" else 2 * HALF + h * 128
                            nc.tensor.matmul(
                                ps[h][:, r0:r1],
                                bias_sb[0:1, bcol:bcol + 128],
                                ones_sb[0:1, 0:r1 - r0],
                                start=False, stop=False,
                            )
                rt = rt_ap[c]
                wc = wc_ap[c]
                for h in range(2):
                    for mi, (kind, r0, r1) in enumerate(MOVES):
                        # stationary: wsub half for A rows, wp half for B rows
                        wcol = h * 128 if kind == "A" else 2 * HALF + h * 128
                        mm = nc.tensor.matmul(
                            ps[h][:, r0:r1],
                            wc[:, wcol:wcol + 128],
                            rt[:, r0:r1],
                            start=(c == 0 and kind == "A"),
                            stop=(c == KC - 1 and mi != 2),
                        )
                        # start=True clears the whole bank; the B range shares
                        # bank 2 with the A range 1024:1152 and must be
                        # ordered after its clear.
                        if c == 0:
                            if kind == "A" and r0 == 1024:
                                bank_clear[h] = mm.ins
                            elif kind == "B":
                                add_dep_helper(mm.ins, bank_clear[h], sync=False,
                                               reason="psum bank-clear before B range")

            # ---- epilogue (all in the transposed [dp, row] layout) ----
            # A: PSUM->SBUF copy per half (fp16 cast), DVE and ACT in
            # parallel, one DMA each
            oA0 = opool.tile([128, RA], fin, tag="oA0")
            nc.vector.tensor_copy(oA0[:], psL[:, 0:RA])
            nc.sync.dma_start(subT_out[0], oA0[:])
            oA1 = opool.tile([128, RA], fin, tag="oA1")
            nc.scalar.copy(oA1[:], psH[:, 0:RA])
            nc.scalar.dma_start(subT_out[1], oA1[:])

            # B: rotary. out_lo = pkL*cos - pkH*sin ; out_hi = pkH*cos + pkL*sin
            pkL = psL[:, RA:RT]
            pkH = psH[:, RA:RT]
            t1 = opool.tile([128, RB], f32, tag="t1")
            t2 = opool.tile([128, RB], f32, tag="t2")
            t3 = opool.tile([128, RB], f32, tag="t3")
            t4 = opool.tile([128, RB], f32, tag="t4")
            o1 = opool.tile([128, RB], f32, tag="o1")
            o2 = opool.tile([128, RB], f32, tag="o2")
            nc.vector.tensor_mul(t1[:], pkL, cos_sb[:])
            nc.vector.tensor_mul(t2[:], pkH, sin_sb[:])
            nc.vector.tensor_sub(o1[:], t1[:], t2[:])
            nc.sync.dma_start(pkT_out[0], o1[:])
            nc.vector.tensor_mul(t3[:], pkH, cos_sb[:])
            nc.vector.tensor_mul(t4[:], pkL, sin_sb[:])
            nc.vector.tensor_add(o2[:], t3[:], t4[:])
            nc.scalar.dma_start(pkT_out[1], o2[:])

    nc.compile()
    return nc


def _get_nc():
    if "nc" not in _CACHE:
        _CACHE["nc"] = _build_nc()
    return _CACHE["nc"]


def kernel(x, vit_features, token_pooling, is_image_token,
           is_indexable_image_token, image_features_mask,
           W_patch_k, b_patch_k, W_subpatch_k, b_subpatch_k,
           no_point_vector, trace=False):
    global LAST_RESULTS
    from concourse.bass_utils import run_bass_kernel_spmd

    x = np.asarray(x, dtype=np.float32)
    vit = np.asarray(vit_features, dtype=np.float32)
    is_image_token = np.asarray(is_image_token, dtype=bool)
    is_indexable = np.asarray(is_indexable_image_token, dtype=bool)
    image_features_mask = np.asarray(image_features_mask, dtype=bool)
    W_patch_k = np.asarray(W_patch_k, dtype=np.float32)
    b_patch_k = np.asarray(b_patch_k, dtype=np.float32)
    W_subpatch_k = np.asarray(W_subpatch_k, dtype=np.float32)
    b_subpatch_k = np.asarray(b_subpatch_k, dtype=np.float32)
    no_point_vector = np.asarray(no_point_vector, dtype=np.float32)

    # ---- host-side index plumbing (exact replication of the reference) ----
    src_mask = is_image_token.reshape(-1)
    dst_mask = image_features_mask.reshape(-1)
    order = np.argsort(~src_mask, kind="stable")
    dst_rank = np.cumsum(dst_mask.astype(np.int32)) - 1
    take = order[dst_rank]                               # [B*N]
    pos = np.cumsum(is_indexable.astype(np.int32), axis=-1) - 1   # [B,S]
    pos_sel = pos.reshape(-1)[take]                      # [B*N]

    inv_freq = (1.0 / (np.float32(THETA) **
                       (np.arange(0, DP, 2, dtype=np.float32) / np.float32(DP))))
    freqs = inv_freq[:, None].astype(np.float32) * pos_sel[None, :].astype(np.float32)
    cos_t = np.cos(freqs).astype(np.float32)             # [128, B*N] transposed
    sin_t = np.sin(freqs).astype(np.float32)

    # ---- shard + lay out device inputs ----
    in_np = np.float16 if IN_DTYPE == "f16" else np.float32
    scale = np.float32(1.0 / np.sqrt(np.float32(D)))
    wc = np.empty((D, 2 * DP), dtype=in_np)
    wc[:, :DP] = W_subpatch_k.T
    wc[:, DP:] = (W_patch_k * scale).T
    wcT = wc.reshape(KC, 128, 2 * DP)
    biasf = np.concatenate([b_subpatch_k, b_patch_k]).astype(in_np).reshape(1, 2 * DP)

    vit_flat = vit.reshape(B * N * P, DV)
    xg = x.reshape(B * S, D)[take]                       # [B*N, D]

    in_maps = []
    for i in range(NCORES):
        rowsT_i = np.empty((D, RT), dtype=in_np)
        rowsT_i[:, :RA] = vit_flat[i * RA:(i + 1) * RA].T
        rowsT_i[:, RA:] = xg[i * RB:(i + 1) * RB].T
        in_maps.append(dict(
            rowsT=rowsT_i.reshape(KC, 128, RT), wcT=wcT, biasf=biasf,
            cosTt=np.ascontiguousarray(cos_t[:, i * RB:(i + 1) * RB]),
            sinTt=np.ascontiguousarray(sin_t[:, i * RB:(i + 1) * RB]),
        ))

    nc = _get_nc()
    bkr = run_bass_kernel_spmd(nc, in_maps, list(range(NCORES)), trace=trace)
    LAST_RESULTS = bkr
    results = bkr.results

    # device outputs are [dp, row]; transpose back while assembling
    sub = np.concatenate(
        [results[i]["subT_out"].reshape(DP, RA).T.astype(np.float32) for i in range(NCORES)], axis=0)
    pk = np.concatenate(
        [results[i]["pkT_out"].reshape(DP, RB).T for i in range(NCORES)], axis=0)

    # ---- assemble outputs on host ----
    pk[~dst_mask] = 0.0
    patch_k = np.concatenate(
        [pk.reshape(B, N, DP),
         np.broadcast_to(no_point_vector, (B, 1, DP))], axis=1).astype(np.float32)
    pkm = np.where(dst_mask, is_indexable.reshape(-1)[take], False).reshape(B, N)
    patch_k_mask = np.concatenate([pkm, np.ones((B, 1), bool)], axis=1)
    subpatch_k = sub.reshape(B, N, P, DP)
    image_pos_ids = np.where(dst_mask, pos_sel, 0).reshape(B, N).astype(np.int32)
    return patch_k, patch_k_mask, subpatch_k, image_pos_ids


# revision 28
# speedup vs baseline: 1.1015x; 1.1015x over previous
"""Trainium2 Bass kernel for nn_PointPredictor (Molmo point-predictor head).

Strategy
--------
Both heavy ops are row-parallel [R, 2048] @ [2048, 256] matmuls:
  * subpatch_k: vit_features rows (B*N*P = 9216 rows)
  * patch_k:    the B*N = 2304 compacted/selected rows of x (the masked
    compaction is a pure gather, so only the selected rows are computed)
Rows are sharded evenly across the 8 NeuronCores (1152 + 288 = 1440 rows
per core); the small weights are replicated. The host computes the
gather indices / positions / cos-sin rotary tables (tiny mask math),
transposes the row blocks so the contraction dim lands on SBUF
partitions, and assembles/re-transposes the outputs.

On device (per core, per K-chunk of 128): the weight chunk halves
[128, 128] are the PE stationary operand and the 1440 rows stream as the
moving operand, producing transposed outputs [dp_half, row] in two
3-bank PSUM tiles (fp32 accumulation over 16 chunks). Operands stream
as fp16 (IN_DTYPE); bias lands as a K=1 rank-1 matmul; rotary runs on
the vector engine in fp32 in the transposed layout (host supplies
transposed cos/sin tables). Outputs DMA out as [dp, row] and the host
transposes back while assembling.

PSUM bank note: start=True clears has_written for the WHOLE 2KB bank,
so only the very first matmul into each bank uses start=True (explicitly
ordered first); later writers to other column ranges of the bank rely on
per-element has_written bits (overwrite-on-first-touch, accumulate
after).
"""

import numpy as np

B, S, N, P, D, DP, DV = 4, 2048, 576, 4, 2048, 256, 2048
THETA = 10000.0
NCORES = 8
RA = (B * N * P) // NCORES      # 1152 subpatch rows per core
RB = (B * N) // NCORES          # 288 patch rows per core
RT = RA + RB                    # 1440 rows per core
KC = D // 128                   # 16 contraction chunks
HALF = DP // 2                  # 128

# moving-operand ranges: A rows in <=512 slices (one PSUM bank each),
# then the B rows (1152:1440, inside bank 2)
MOVES = [("A", 0, 512), ("A", 512, 1024), ("A", 1024, 1152), ("B", 1152, 1440)]

_CACHE = {}
LAST_RESULTS = None  # BassKernelResults of the most recent run (for test.py)

# Input dtype for the matmul operands (row data + weights).
# "f16": half the DMA traffic, ~3e-4 scaled error. "f32r": ~1.5e-4 error.
IN_DTYPE = "f16"


def _build_nc():
    import concourse.bacc as bacc
    import concourse.mybir as mybir
    import concourse.tile as tile
    from concourse.tile_rust import add_dep_helper

    f32 = mybir.dt.float32
    fin = mybir.dt.float16 if IN_DTYPE == "f16" else mybir.dt.float32r

    nc = bacc.Bacc("TRN2", target_bir_lowering=False, debug=False)
    rowsT = nc.dram_tensor("rowsT", [KC, 128, RT], fin, kind="ExternalInput").ap()
    wcT = nc.dram_tensor("wcT", [KC, 128, 2 * DP], fin, kind="ExternalInput").ap()
    biasf = nc.dram_tensor("biasf", [1, 2 * DP], fin, kind="ExternalInput").ap()
    cosTt = nc.dram_tensor("cosTt", [128, RB], f32, kind="ExternalInput").ap()
    sinTt = nc.dram_tensor("sinTt", [128, RB], f32, kind="ExternalInput").ap()
    subT_out = nc.dram_tensor("subT_out", [2, 128, RA], f32, kind="ExternalOutput").ap()
    pkT_out = nc.dram_tensor("pkT_out", [2, 128, RB], f32, kind="ExternalOutput").ap()
    warm_out = nc.dram_tensor("warm_out", [1, 1], f32, kind="ExternalOutput").ap()

    PSW = 1536  # 3 banks of 512 fp32, covers the 1440 row columns

    with tile.TileContext(nc) as tc:
        with (
            tc.tile_pool(name="const", bufs=1) as kpool,
            tc.tile_pool(name="chunks", bufs=6) as cpool,
            tc.tile_pool(name="psacc", bufs=1, space="PSUM") as ppool,
            tc.tile_pool(name="outs", bufs=2) as opool,
        ):
            # transposed output accumulators: ps[h][dp_lo+p, row]
            psL = ppool.tile([128, PSW], f32, name="psL", tag="psL")
            psH = ppool.tile([128, PSW], f32, name="psH", tag="psH")
            ps = [psL, psH]

            ones_sb = kpool.tile([1, 512], fin, tag="ones")
            nc.vector.memset(ones_sb[:], 1.0)
            bias_sb = kpool.tile([1, 2 * DP], fin, tag="biasf")
            cos_sb = kpool.tile([128, RB], f32, tag="cos")
            sin_sb = kpool.tile([128, RB], f32, tag="sin")

            # Warm-up matmuls: ~3.4us of PE activity releases the HAM clock
            # gate (1.2 -> 2.4 GHz) before the first real matmul arrives.
            ps_warm = ppool.tile([128, 512], f32, name="ps_warm", tag="ps_warm")
            for w in range(18):
                nc.tensor.matmul(ps_warm[0:128, 0:128], ones_sb[0:1, 0:128],
                                 ones_sb[0:1, 0:128], start=True, stop=True)
            junk_sb = kpool.tile([1, 1], f32, tag="junk")
            nc.vector.tensor_copy(junk_sb[:], ps_warm[0:1, 0:1])
            nc.sync.dma_start(warm_out, junk_sb[:])

            # Per-chunk DMAs, alternating between the two HWDGE rings
            # (sync + scalar); rows and weights take opposite rings so each
            # chunk's pair lands in parallel. Constants queue after all
            # chunk loads (they are only needed by the tail).
            rt_ap = {}
            wc_ap = {}
            for c in range(KC):
                rte = nc.sync if c % 2 == 0 else nc.scalar
                wce = nc.scalar if c % 2 == 0 else nc.sync
                t = cpool.tile([128, RT], fin, name=f"rtc{c}", tag="rt", bufs=8)
                rte.dma_start(t[:], rowsT[c])
                rt_ap[c] = t[:]
                w = cpool.tile([128, 2 * DP], fin, name=f"wcc{c}", tag="wc", bufs=8)
                wce.dma_start(w[:], wcT[c])
                wc_ap[c] = w[:]
            nc.scalar.dma_start(bias_sb[:], biasf)
            nc.scalar.dma_start(cos_sb[:], cosTt)
            nc.scalar.dma_start(sin_sb[:], sinTt)

            bank_clear = {}
            for c in range(KC):
                if c == 12:
                    # biases as K=1 rank-1 accumulation (bias_half ⊗ ones),
                    # slotted mid-chain after the bias DMA has landed
                    for h in range(2):
                        for kind, r0, r1 in MOVES:
                            bcol = h * 128 if kind == "A" else 2 * HALF + h * 128
                            nc.tensor.matmul(
                                ps[h][:, r0:r1],
                                bias_sb[0:1, bcol:bcol + 128],
                                ones_sb[0:1, 0:r1 - r0],
                                start=False, stop=False,
                            )
                rt = rt_ap[c]
                wc = wc_ap[c]
                for h in range(2):
                    for mi, (kind, r0, r1) in enumerate(MOVES):
                        # stationary: wsub half for A rows, wp half for B rows
                        wcol = h * 128 if kind == "A" else 2 * HALF + h * 128
                        mm = nc.tensor.matmul(
                            ps[h][:, r0:r1],
                            wc[:, wcol:wcol + 128],
                            rt[:, r0:r1],
                            start=(c == 0 and kind == "A"),
                            stop=(c == KC - 1 and mi != 2),
                        )
                        # start=True clears the whole bank; the B range shares
                        # bank 2 with the A range 1024:1152 and must be
                        # ordered after its clear.
                        if c == 0:
                            if kind == "A" and r0 == 1024:
                                bank_clear[h] = mm.ins
                            elif kind == "B":
                                add_dep_helper(mm.ins, bank_clear[h], sync=False,
                                               reason="psum bank-clear before B range")

            # ---- epilogue (all in the transposed [dp, row] layout) ----
            # A: PSUM->SBUF copy per half, one DMA each
            for h in range(2):
                o = opool.tile([128, RA], f32, tag="oA")
                nc.vector.tensor_copy(o[:], ps[h][:, 0:RA])
                (nc.sync if h == 0 else nc.scalar).dma_start(subT_out[h], o[:])

            # B: rotary. out_lo = pkL*cos - pkH*sin ; out_hi = pkH*cos + pkL*sin
            pkL = psL[:, RA:RT]
            pkH = psH[:, RA:RT]
            t1 = opool.tile([128, RB], f32, tag="t1")
            t2 = opool.tile([128, RB], f32, tag="t2")
            t3 = opool.tile([128, RB], f32, tag="t3")
            t4 = opool.tile([128, RB], f32, tag="t4")
            o1 = opool.tile([128, RB], f32, tag="o1")
            o2 = opool.tile([128, RB], f32, tag="o2")
            nc.vector.tensor_mul(t1[:], pkL, cos_sb[:])
            nc.vector.tensor_mul(t2[:], pkH, sin_sb[:])
            nc.vector.tensor_sub(o1[:], t1[:], t2[:])
            nc.sync.dma_start(pkT_out[0], o1[:])
            nc.vector.tensor_mul(t3[:], pkH, cos_sb[:])
            nc.vector.tensor_mul(t4[:], pkL, sin_sb[:])
            nc.vector.tensor_add(o2[:], t3[:], t4[:])
            nc.scalar.dma_start(pkT_out[1], o2[:])

    nc.compile()
    return nc


def _get_nc():
    if "nc" not in _CACHE:
        _CACHE["nc"] = _build_nc()
    return _CACHE["nc"]


def kernel(x, vit_features, token_pooling, is_image_token,
           is_indexable_image_token, image_features_mask,
           W_patch_k, b_patch_k, W_subpatch_k, b_subpatch_k,
           no_point_vector, trace=False):
    global LAST_RESULTS
    from concourse.bass_utils import run_bass_kernel_spmd

    x = np.asarray(x, dtype=np.float32)
    vit = np.asarray(vit_features, dtype=np.float32)
    is_image_token = np.asarray(is_image_token, dtype=bool)
    is_indexable = np.asarray(is_indexable_image_token, dtype=bool)
    image_features_mask = np.asarray(image_features_mask, dtype=bool)
    W_patch_k = np.asarray(W_patch_k, dtype=np.float32)
    b_patch_k = np.asarray(b_patch_k, dtype=np.float32)
    W_subpatch_k = np.asarray(W_subpatch_k, dtype=np.float32)
    b_subpatch_k = np.asarray(b_subpatch_k, dtype=np.float32)
    no_point_vector = np.asarray(no_point_vector, dtype=np.float32)

    # ---- host-side index plumbing (exact replication of the reference) ----
    src_mask = is_image_token.reshape(-1)
    dst_mask = image_features_mask.reshape(-1)
    order = np.argsort(~src_mask, kind="stable")
    dst_rank = np.cumsum(dst_mask.astype(np.int32)) - 1
    take = order[dst_rank]                               # [B*N]
    pos = np.cumsum(is_indexable.astype(np.int32), axis=-1) - 1   # [B,S]
    pos_sel = pos.reshape(-1)[take]                      # [B*N]

    inv_freq = (1.0 / (np.float32(THETA) **
                       (np.arange(0, DP, 2, dtype=np.float32) / np.float32(DP))))
    freqs = inv_freq[:, None].astype(np.float32) * pos_sel[None, :].astype(np.float32)
    cos_t = np.cos(freqs).astype(np.float32)             # [128, B*N] transposed
    sin_t = np.sin(freqs).astype(np.float32)

    # ---- shard + lay out device inputs ----
    in_np = np.float16 if IN_DTYPE == "f16" else np.float32
    scale = np.float32(1.0 / np.sqrt(np.float32(D)))
    wc = np.empty((D, 2 * DP), dtype=in_np)
    wc[:, :DP] = W_subpatch_k.T
    wc[:, DP:] = (W_patch_k * scale).T
    wcT = wc.reshape(KC, 128, 2 * DP)
    biasf = np.concatenate([b_subpatch_k, b_patch_k]).astype(in_np).reshape(1, 2 * DP)

    vit_flat = vit.reshape(B * N * P, DV)
    xg = x.reshape(B * S, D)[take]                       # [B*N, D]

    in_maps = []
    for i in range(NCORES):
        rowsT_i = np.empty((D, RT), dtype=in_np)
        rowsT_i[:, :RA] = vit_flat[i * RA:(i + 1) * RA].T
        rowsT_i[:, RA:] = xg[i * RB:(i + 1) * RB].T
        in_maps.append(dict(
            rowsT=rowsT_i.reshape(KC, 128, RT), wcT=wcT, biasf=biasf,
            cosTt=np.ascontiguousarray(cos_t[:, i * RB:(i + 1) * RB]),
            sinTt=np.ascontiguousarray(sin_t[:, i * RB:(i + 1) * RB]),
        ))

    nc = _get_nc()
    bkr = run_bass_kernel_spmd(nc, in_maps, list(range(NCORES)), trace=trace)
    LAST_RESULTS = bkr
    results = bkr.results

    # device outputs are [dp, row]; transpose back while assembling
    sub = np.concatenate(
        [results[i]["subT_out"].reshape(DP, RA).T for i in range(NCORES)], axis=0)
    pk = np.concatenate(
        [results[i]["pkT_out"].reshape(DP, RB).T for i in range(NCORES)], axis=0)

    # ---- assemble outputs on host ----
    pk[~dst_mask] = 0.0
    patch_k = np.concatenate(
        [pk.reshape(B, N, DP),
         np.broadcast_to(no_point_vector, (B, 1, DP))], axis=1).astype(np.float32)
    pkm = np.where(dst_mask, is_indexable.reshape(-1)[take], False).reshape(B, N)
    patch_k_mask = np.concatenate([pkm, np.ones((B, 1), bool)], axis=1)
    subpatch_k = sub.reshape(B, N, P, DP)
    image_pos_ids = np.where(dst_mask, pos_sel, 0).reshape(B, N).astype(np.int32)
    return patch_k, patch_k_mask, subpatch_k, image_pos_ids
